# revision 1
# baseline (speedup 1.0000x reference)
"""Trainium2 Bass kernel for nn_ContinuousConvolutionBlock (gnn_message_passing).

Strategy (per sharding hint: partition points across 8 cores; each core owns its
queries' scatter-reduce and tap-GEMM; filter + dense weights replicated):

Host side (index plumbing / input marshalling only — zero FLOPs):
  - qry_idx is sorted; queries are grouped into 8-query blocks, blocks paired
    into 128-edge-slot "chunks" (two-pointer bin packing, ~3% padding).
  - Consecutive block ranges are assigned to the 8 cores; per-core per-slot
    payload arrays (pos[src], pos[qry], feats[src], local query id) are
    marshalled on host and DMA'd in dense [128 x NCH x k] layout.

Device side (all FLOP-bearing compute):
  - Geometry: ball->cube volume-preserving map + trilinear corner weights
    (DVE arithmetic + ACT sqrt/arctan/sign/abs), producing per-slot 4-wide
    one-hot weight vectors w4x/w4y/w4z (separable trilinear factorization).
  - Scatter-reduce as factored matmul per chunk: with R[slot,(ax,c)] =
    w4x (x) feats and L[slot,(q,az,ay)] = Qoh (x) w4z (x) w4y, PE computes
    A^T[(ax,c),(q,az,ay)] = R^T @ L, accumulating the per-query tap grid
    A[q, az,ay,ax, c] directly in transposed layout (PSUM).
  - Tap-GEMM: for each (az,ay) tap-pair t, out^T += G_t^T @ A^T-slices,
    accumulated over 16 t in PSUM. G is the filter regrouped on host
    (pure relayout, replicated to all cores).
  - Dense branch: out_dense^T = dense_w^T @ feats^T + b on PE.
  Outputs are produced transposed ([64, nq]); host transposes/reorders back.
"""
import sys
import os
sys.path.insert(0, '/opt/trn_rl_repo')
import numpy as np

N = 30000
CIN = 32
COUT = 64
KS = 4
EXTENT = 0.08
NCORES = 8
NBLK = N // 8  # 3750 eight-query blocks

_COMPILED = {}


# ----------------------------------------------------------------------------
# Host planning
# ----------------------------------------------------------------------------
def _plan(qry_idx):
    deg = np.bincount(qry_idx, minlength=N)
    bsz = deg.reshape(NBLK, 8).sum(1)
    bstart = np.concatenate([[0], np.cumsum(bsz)]).astype(np.int64)
    per = [NBLK // NCORES + (1 if c < NBLK % NCORES else 0) for c in range(NCORES)]
    b0 = np.concatenate([[0], np.cumsum(per)]).astype(np.int64)
    plans = []
    for c in range(NCORES):
        blocks = list(range(b0[c], b0[c + 1]))
        asc = sorted(blocks, key=lambda b: bsz[b])
        chunks = []
        lo, hi = 0, len(asc) - 1
        while lo <= hi:
            if lo == hi:
                chunks.append((asc[hi], None)); break
            if bsz[asc[hi]] + bsz[asc[lo]] <= 128:
                chunks.append((asc[hi], asc[lo])); hi -= 1; lo += 1
            else:
                chunks.append((asc[hi], None)); hi -= 1
        plans.append(dict(blocks=blocks, chunks=chunks, q0=int(8 * b0[c]),
                          nq=int(8 * (b0[c + 1] - b0[c]))))
    return plans, bstart, bsz


def _pack_core(plan_c, bstart, pos, feats, qry_idx, src_idx, NCHP):
    """Build per-slot payload arrays in [128, NCHP, k] layout."""
    possrc = np.zeros((128, NCHP, 4), np.float32)
    posqry = np.zeros((128, NCHP, 4), np.float32)
    fsrc = np.zeros((128, NCHP, CIN), np.float32)
    qlocf = np.full((128, NCHP), -1.0, np.float32)
    for ci, (bA, bB) in enumerate(plan_c['chunks']):
        s = 0
        for half, b in enumerate((bA, bB)):
            if b is None:
                continue
            e0, e1 = int(bstart[b]), int(bstart[b + 1])
            n = e1 - e0
            sl = slice(s, s + n)
            possrc[sl, ci, 0:3] = pos[src_idx[e0:e1]]
            posqry[sl, ci, 0:3] = pos[qry_idx[e0:e1]]
            fsrc[sl, ci, :] = feats[src_idx[e0:e1]]
            qlocf[sl, ci] = (qry_idx[e0:e1] - 8 * b) + 8 * half
            s += n
    return possrc, posqry, fsrc, qlocf


# ----------------------------------------------------------------------------
# Device kernel
# ----------------------------------------------------------------------------
def _build_bass(NCHP, NQ):
    import concourse.bass as bass
    import concourse.tile as tile
    from concourse import bacc, mybir
    from concourse.bass import AP

    f32 = mybir.dt.float32
    f32r = mybir.dt.float32r
    i32 = mybir.dt.int32
    ALU = mybir.AluOpType
    ACT = mybir.ActivationFunctionType
    EPS = 1e-12
    F4PI = float(4.0 / np.pi)

    nc = bacc.Bacc("TRN2", target_bir_lowering=False, debug=False)

    possrc = nc.dram_tensor("possrc", (128, NCHP, 4), f32, kind="ExternalInput")
    posqry = nc.dram_tensor("posqry", (128, NCHP, 4), f32, kind="ExternalInput")
    fsrc = nc.dram_tensor("fsrc", (128, NCHP, CIN), f32, kind="ExternalInput")
    qlocf = nc.dram_tensor("qlocf", (128, NCHP), f32, kind="ExternalInput")
    g2 = nc.dram_tensor("g2", (128, 16 * 64), f32, kind="ExternalInput")
    featsT = nc.dram_tensor("featsT", (CIN, NQ), f32, kind="ExternalInput")
    denw = nc.dram_tensor("denw", (CIN, COUT), f32, kind="ExternalInput")
    denb = nc.dram_tensor("denb", (COUT, 1), f32, kind="ExternalInput")

    outconvT = nc.dram_tensor("outconvT", (COUT, NQ), f32, kind="ExternalOutput")
    outdenseT = nc.dram_tensor("outdenseT", (COUT, NQ), f32, kind="ExternalOutput")

    W = NCHP            # geometry tile width (all chunks at once)
    NGRP = NCHP // 16   # tap-GEMM groups

    with tile.TileContext(nc) as tc:
        with tc.tile_pool(name="inp", bufs=1) as inp, \
             tc.tile_pool(name="geo", bufs=1) as geo, \
             tc.tile_pool(name="tmp", bufs=1) as tmp, \
             tc.tile_pool(name="lr", bufs=10) as lrp, \
             tc.tile_pool(name="at", bufs=3) as atp, \
             tc.tile_pool(name="outp", bufs=4) as outp, \
             tc.tile_pool(name="ps1", bufs=4, space="PSUM") as ps1, \
             tc.tile_pool(name="ps2", bufs=2, space="PSUM") as ps2:

            # ---------------- input DMAs ----------------
            t_ps = inp.tile([128, W, 4], f32)
            t_pq = inp.tile([128, W, 4], f32)
            t_f = inp.tile([128, W, CIN], f32)
            t_ql = inp.tile([128, W], f32)
            t_g2 = inp.tile([128, 16 * 64], f32)
            t_ftT = inp.tile([CIN, NQ], f32)
            t_dw = inp.tile([CIN, COUT], f32)
            t_db = inp.tile([COUT, 1], f32)
            nc.sync.dma_start(t_ps[:], possrc[:])
            nc.sync.dma_start(t_pq[:], posqry[:])
            nc.sync.dma_start(t_f[:], fsrc[:])
            nc.sync.dma_start(t_ql[:], qlocf[:])
            nc.sync.dma_start(t_g2[:], g2[:])
            nc.sync.dma_start(t_ftT[:], featsT[:])
            nc.sync.dma_start(t_dw[:], denw[:])
            nc.sync.dma_start(t_db[:], denb[:])

            # round filter to f32r once
            t_g2r = inp.tile([128, 16 * 64], f32r)
            nc.vector.tensor_copy(t_g2r[:], t_g2[:])

            # iota constants
            io4i = tmp.tile([128, 4], i32)
            nc.gpsimd.iota(io4i[:], pattern=[[1, 4]], base=0, channel_multiplier=0)
            io4 = geo.tile([128, 4], f32)
            nc.vector.tensor_copy(io4[:], io4i[:])
            io16i = tmp.tile([128, 16], i32)
            nc.gpsimd.iota(io16i[:], pattern=[[1, 16]], base=0, channel_multiplier=0)
            io16 = geo.tile([128, 16], f32)
            nc.vector.tensor_copy(io16[:], io16i[:])

            # ---------------- geometry ----------------
            _tn = [0]
            _free_tags = []
            _tag_of = {}

            _seq = [0]

            def T(shape=(128, W), dt_=f32):
                if _free_tags:
                    tg = _free_tags.pop()
                else:
                    _tn[0] += 1
                    tg = f"t{_tn[0]}"
                _seq[0] += 1
                t = tmp.tile(list(shape), dt_, name=f"{tg}_u{_seq[0]}", tag=tg)
                _tag_of[id(t)] = tg
                return t

            def F(*ts):
                for t in ts:
                    _free_tags.append(_tag_of.pop(id(t)))

            TT = nc.vector.tensor_tensor
            TS = nc.vector.tensor_scalar
            STT = nc.vector.scalar_tensor_tensor

            # r = (ps - pq) * (2/EXTENT), per coord [128, W, 3]
            r = T((128, W, 3))
            TT(out=r[:], in0=t_ps[:, :, 0:3], in1=t_pq[:, :, 0:3], op=ALU.subtract)
            rs = T((128, W, 3))
            TS(rs[:], r[:], float(2.0 / EXTENT), None, op0=ALU.mult)
            F(r)
            x, y, z = rs[:, :, 0], rs[:, :, 1], rs[:, :, 2]

            sq3 = T((128, W, 3))
            TT(out=sq3[:], in0=rs[:], in1=rs[:], op=ALU.mult)
            x2, y2, z2 = sq3[:, :, 0], sq3[:, :, 1], sq3[:, :, 2]
            xy2 = T()
            TT(out=xy2[:], in0=x2, in1=y2, op=ALU.add)
            sq = T()
            TT(out=sq[:], in0=xy2[:], in1=z2, op=ALU.add)

            norm = T()
            nc.scalar.activation(norm[:], sq[:], ACT.Sqrt)
            nxy = T()
            nc.scalar.activation(nxy[:], xy2[:], ACT.Sqrt)

            p125 = T()
            TS(p125[:], z2, 1.25, None, op0=ALU.mult)
            pole = T()
            TT(out=pole[:], in0=p125[:], in1=xy2[:], op=ALU.is_gt)
            F(sq3, xy2, p125)

            azn = T()
            nc.scalar.activation(azn[:], z, ACT.Abs)
            den1 = T()
            STT(out=den1[:], in0=azn[:], scalar=EPS, in1=norm[:], op0=ALU.add, op1=ALU.add)
            rd1 = T()
            nc.vector.reciprocal(rd1[:], den1[:])
            t1s = T()
            STT(out=t1s[:], in0=norm[:], scalar=3.0, in1=rd1[:], op0=ALU.mult, op1=ALU.mult)
            s1 = T()
            nc.scalar.activation(s1[:], t1s[:], ACT.Sqrt)
            F(azn, den1, rd1, t1s)

            den2 = T()
            TS(den2[:], nxy[:], EPS, None, op0=ALU.add)
            rd2 = T()
            nc.vector.reciprocal(rd2[:], den2[:])
            s2 = T()
            TT(out=s2[:], in0=norm[:], in1=rd2[:], op=ALU.mult)
            F(nxy, den2, rd2)

            d12 = T()
            TT(out=d12[:], in0=s1[:], in1=s2[:], op=ALU.subtract)
            pw = T()
            TT(out=pw[:], in0=pole[:], in1=d12[:], op=ALU.mult)
            wq = T()
            TT(out=wq[:], in0=s2[:], in1=pw[:], op=ALU.add)
            F(s1, s2, d12, pw)

            xc = T()
            TT(out=xc[:], in0=x, in1=wq[:], op=ALU.mult)
            yc = T()
            TT(out=yc[:], in0=y, in1=wq[:], op=ALU.mult)
            F(wq)

            sgz = T()
            nc.scalar.activation(sgz[:], z, ACT.Sign)
            zcp = T()
            TT(out=zcp[:], in0=sgz[:], in1=norm[:], op=ALU.mult)
            zce = T()
            TS(zce[:], z, 1.5, None, op0=ALU.mult)
            dz = T()
            TT(out=dz[:], in0=zcp[:], in1=zce[:], op=ALU.subtract)
            pz = T()
            TT(out=pz[:], in0=pole[:], in1=dz[:], op=ALU.mult)
            zc = T()
            TT(out=zc[:], in0=zce[:], in1=pz[:], op=ALU.add)
            F(sgz, zcp, zce, dz, pz, pole, norm, rs)

            zero1 = T()
            TS(zero1[:], sq[:], EPS, None, op0=ALU.is_lt)
            onem1 = T()
            TS(onem1[:], zero1[:], -1.0, 1.0, op0=ALU.mult, op1=ALU.add)
            for t_ in (xc, yc, zc):
                TT(out=t_[:], in0=t_[:], in1=onem1[:], op=ALU.mult)
            F(sq, zero1, onem1)

            # cylinder -> cube
            xc2 = T()
            TT(out=xc2[:], in0=xc[:], in1=xc[:], op=ALU.mult)
            yc2 = T()
            TT(out=yc2[:], in0=yc[:], in1=yc[:], op=ALU.mult)
            sqxy = T()
            TT(out=sqxy[:], in0=xc2[:], in1=yc2[:], op=ALU.add)
            nrm = T()
            nc.scalar.activation(nrm[:], sqxy[:], ACT.Sqrt)
            F(xc2, yc2)

            axc = T()
            nc.scalar.activation(axc[:], xc[:], ACT.Abs)
            ayc = T()
            nc.scalar.activation(ayc[:], yc[:], ACT.Abs)
            abr = T()
            TT(out=abr[:], in0=ayc[:], in1=axc[:], op=ALU.is_le)

            mx = T()
            TS(mx[:], axc[:], EPS, None, op0=ALU.is_lt)
            sfx = T()
            TT(out=sfx[:], in0=xc[:], in1=mx[:], op=ALU.add)
            my = T()
            TS(my[:], ayc[:], EPS, None, op0=ALU.is_lt)
            sfy = T()
            TT(out=sfy[:], in0=yc[:], in1=my[:], op=ALU.add)
            F(axc, ayc, mx, my)

            rsx = T()
            nc.vector.reciprocal(rsx[:], sfx[:])
            rsy = T()
            nc.vector.reciprocal(rsy[:], sfy[:])
            ratx = T()
            TT(out=ratx[:], in0=xc[:], in1=rsy[:], op=ALU.mult)
            raty = T()
            TT(out=raty[:], in0=yc[:], in1=rsx[:], op=ALU.mult)
            at1 = T()
            nc.scalar.activation(at1[:], ratx[:], ACT.Arctan)
            at2 = T()
            nc.scalar.activation(at2[:], raty[:], ACT.Arctan)
            F(sfx, sfy, rsx, rsy, ratx, raty)

            sgx = T()
            nc.scalar.activation(sgx[:], xc[:], ACT.Sign)
            sgy = T()
            nc.scalar.activation(sgy[:], yc[:], ACT.Sign)
            tmpa = T()
            TT(out=tmpa[:], in0=sgx[:], in1=nrm[:], op=ALU.mult)
            tmpb = T()
            TT(out=tmpb[:], in0=sgy[:], in1=nrm[:], op=ALU.mult)
            F(sgx, sgy, nrm, xc, yc)

            # xo = where(a, tmpa, tmpb * F4PI * at1)
            xoe = T()
            STT(out=xoe[:], in0=at1[:], scalar=F4PI, in1=tmpb[:], op0=ALU.mult, op1=ALU.mult)
            dxo = T()
            TT(out=dxo[:], in0=tmpa[:], in1=xoe[:], op=ALU.subtract)
            adx = T()
            TT(out=adx[:], in0=abr[:], in1=dxo[:], op=ALU.mult)
            xo = T()
            TT(out=xo[:], in0=xoe[:], in1=adx[:], op=ALU.add)
            # yo = where(a, tmpa * F4PI * at2, tmpb)
            yoe = T()
            STT(out=yoe[:], in0=at2[:], scalar=F4PI, in1=tmpa[:], op0=ALU.mult, op1=ALU.mult)
            dyo = T()
            TT(out=dyo[:], in0=yoe[:], in1=tmpb[:], op=ALU.subtract)
            ady = T()
            TT(out=ady[:], in0=abr[:], in1=dyo[:], op=ALU.mult)
            yo = T()
            TT(out=yo[:], in0=tmpb[:], in1=ady[:], op=ALU.add)
            F(at1, at2, xoe, dxo, adx, yoe, dyo, ady, tmpa, tmpb, abr)

            zero2 = T()
            TS(zero2[:], sqxy[:], EPS, None, op0=ALU.is_lt)
            onem2 = T()
            TS(onem2[:], zero2[:], -1.0, 1.0, op0=ALU.mult, op1=ALU.add)
            TT(out=xo[:], in0=xo[:], in1=onem2[:], op=ALU.mult)
            TT(out=yo[:], in0=yo[:], in1=onem2[:], op=ALU.mult)
            F(sqxy, zero2, onem2)

            # ---------------- corner weights w4 ----------------
            def corners_w4(m_ap, w4_t):
                g = T()
                TS(g[:], m_ap, 1.5, 1.5, op0=ALU.mult, op1=ALU.add)
                gc = T()
                TS(gc[:], g[:], 0.0, None, op0=ALU.max)
                g0i = T(dt_=i32)
                TS(g0i[:], gc[:], 0.5, None, op0=ALU.subtract)  # cast rint => floor
                g0 = T()
                nc.vector.tensor_copy(g0[:], g0i[:])
                fr = T()
                TT(out=fr[:], in0=gc[:], in1=g0[:], op=ALU.subtract)
                i0 = T()
                TS(i0[:], g0[:], 3.0, None, op0=ALU.min)
                i1 = T()
                TS(i1[:], g0[:], 1.0, 3.0, op0=ALU.add, op1=ALU.min)
                # e0/e1 one-hots [128, W, 4]
                e0 = T((128, W, 4))
                TT(out=e0[:],
                   in0=AP(io4.tensor, io4[:].offset, [io4[:].ap[0], [0, W], [1, 4]]),
                   in1=AP(i0.tensor, i0[:].offset, [i0[:].ap[0], [1, W], [0, 4]]),
                   op=ALU.is_equal)
                e1 = T((128, W, 4))
                TT(out=e1[:],
                   in0=AP(io4.tensor, io4[:].offset, [io4[:].ap[0], [0, W], [1, 4]]),
                   in1=AP(i1.tensor, i1[:].offset, [i1[:].ap[0], [1, W], [0, 4]]),
                   op=ALU.is_equal)
                onemf = T()
                TS(onemf[:], fr[:], -1.0, 1.0, op0=ALU.mult, op1=ALU.add)
                TT(out=e0[:], in0=e0[:],
                   in1=AP(onemf.tensor, onemf[:].offset, [onemf[:].ap[0], [1, W], [0, 4]]),
                   op=ALU.mult)
                TT(out=e1[:], in0=e1[:],
                   in1=AP(fr.tensor, fr[:].offset, [fr[:].ap[0], [1, W], [0, 4]]),
                   op=ALU.mult)
                TT(out=w4_t[:], in0=e0[:], in1=e1[:], op=ALU.add)
                F(g, gc, g0i, g0, fr, i0, i1, e0, e1, onemf)

            w4x = geo.tile([128, W, 4], f32)
            w4y = geo.tile([128, W, 4], f32)
            w4z = geo.tile([128, W, 4], f32)
            corners_w4(xo[:], w4x)
            corners_w4(yo[:], w4y)
            corners_w4(zc[:], w4z)
            F(xo, yo, zc)

            # Qoh16 [128, W, 16], ZY [128, W, 16]
            qoh = geo.tile([128, W, 16], f32)
            TT(out=qoh[:],
               in0=AP(t_ql.tensor, t_ql[:].offset, [t_ql[:].ap[0], [1, W], [0, 16]]),
               in1=AP(io16.tensor, io16[:].offset, [io16[:].ap[0], [0, W], [1, 16]]),
               op=ALU.is_equal)
            zy = geo.tile([128, W, 16], f32)
            TT(out=zy[:],
               in0=AP(w4z.tensor, w4z[:].offset,
                      [w4z[:].ap[0], [4, W], [1, 4], [0, 4]]),
               in1=AP(w4y.tensor, w4y[:].offset,
                      [w4y[:].ap[0], [4, W], [0, 4], [1, 4]]),
               op=ALU.mult)

            # ---------------- stage-1 + tap-GEMM ----------------
            for g in range(NGRP):
                at_st = atp.tile([128, 16 * 256], f32r, tag="at")
                for cl in range(0, 16, 2):
                    ps_t = ps1.tile([128, 512], f32, space="PSUM", tag="s1")
                    for par in range(2):
                        ci = g * 16 + cl + par
                        # R [128, (ax, c)]
                        R = lrp.tile([128, 128], f32r, tag="R")
                        wx = w4x[:, ci, :]
                        ff = t_f[:, ci, :]
                        TT(out=AP(R.tensor, R[:].offset, [R[:].ap[0], [32, 4], [1, 32]]),
                           in0=AP(wx.tensor, wx.offset, [wx.ap[0], [1, 4], [0, 32]]),
                           in1=AP(ff.tensor, ff.offset, [ff.ap[0], [0, 4], [1, 32]]),
                           op=ALU.mult)
                        # L [128, (half, q, t)]
                        L = lrp.tile([128, 256], f32r, tag="L")
                        qq = qoh[:, ci, :]
                        zz = zy[:, ci, :]
                        TT(out=AP(L.tensor, L[:].offset,
                                  [L[:].ap[0], [128, 2], [16, 8], [1, 16]]),
                           in0=AP(qq.tensor, qq.offset,
                                  [qq.ap[0], [8, 2], [1, 8], [0, 16]]),
                           in1=AP(zz.tensor, zz.offset,
                                  [zz.ap[0], [0, 2], [0, 8], [1, 16]]),
                           op=ALU.mult)
                        nc.tensor.matmul(
                            out=ps_t[:, par * 256:(par + 1) * 256],
                            lhsT=R[:], rhs=L[:], start=True, stop=True)
                    # copy 2 chunks at once, alternating DVE/ACT
                    dst = at_st[:, cl * 256:(cl + 2) * 256]
                    if (cl // 2) % 2 == 0:
                        nc.vector.tensor_copy(dst, ps_t[:])
                    else:
                        nc.scalar.copy(dst, ps_t[:])
                # tap-GEMM for this group
                po = ps2.tile([COUT, 256], f32, space="PSUM", tag="tap")
                for t in range(16):
                    rhs = AP(at_st.tensor, at_st[:].offset + t,
                             [at_st[:].ap[0], [256, 16], [128, 2], [16, 8]])
                    nc.tensor.matmul(
                        out=po[:],
                        lhsT=t_g2r[:, t * 64:(t + 1) * 64],
                        rhs=rhs,
                        start=(t == 0), stop=(t == 15))
                ost = outp.tile([COUT, 256], f32, tag="ocst")
                nc.vector.tensor_copy(ost[:], po[:])
                nc.sync.dma_start(outconvT[:, g * 256:(g + 1) * 256], ost[:])

            # ---------------- dense branch (plain fp32 matmul) ----------------
            NSEG = (NQ + 511) // 512
            for s in range(NSEG):
                j0 = s * 512
                j1 = min(NQ, j0 + 512)
                pd = ps2.tile([COUT, 512], f32, space="PSUM", tag="den")
                nc.tensor.matmul(
                    out=pd[:, 0:j1 - j0],
                    lhsT=t_dw[:],
                    rhs=t_ftT[:, j0:j1],
                    start=True, stop=True)
                db = t_db[:, 0:1]
                odt = outp.tile([COUT, 512], f32, tag="odst")
                TT(out=odt[:, 0:j1 - j0], in0=pd[:, 0:j1 - j0],
                   in1=AP(db.tensor, db.offset, [db.ap[0], [0, j1 - j0]]),
                   op=ALU.add)
                nc.sync.dma_start(outdenseT[:, j0:j1], odt[:, 0:j1 - j0])

    nc.compile()
    return nc


# ----------------------------------------------------------------------------
# Entry point
# ----------------------------------------------------------------------------
def kernel(feats, pos, filt, dense_w, dense_b, src_idx, qry_idx):
    from concourse.bass_utils import run_bass_kernel_spmd

    feats = np.ascontiguousarray(np.asarray(feats, np.float32))
    pos = np.ascontiguousarray(np.asarray(pos, np.float32))
    filt = np.asarray(filt, np.float32)
    dense_w = np.asarray(dense_w, np.float32)
    dense_b = np.asarray(dense_b, np.float32)
    src_idx = np.asarray(src_idx).astype(np.int64)
    qry_idx = np.asarray(qry_idx).astype(np.int64)

    plans, bstart, bsz = _plan(qry_idx)
    NCH = max(len(p['chunks']) for p in plans)
    NCHP = ((NCH + 15) // 16) * 16
    NQ = NCHP * 16

    # filter regroup: G2[ax*32+c, t*64+o] = filt[az, ay, ax, c, o], t = az*4+ay
    G2 = np.zeros((128, 16 * 64), np.float32)
    for az in range(4):
        for ay in range(4):
            t = az * 4 + ay
            for ax in range(4):
                G2[ax * 32:(ax + 1) * 32, t * 64:(t + 1) * 64] = filt[az, ay, ax]

    in_maps = []
    for c, p in enumerate(plans):
        possrc, posqry, fsrc, qlocf = _pack_core(p, bstart, pos, feats,
                                                 qry_idx, src_idx, NCHP)
        ftT = np.zeros((CIN, NQ), np.float32)
        ftT[:, 0:p['nq']] = feats[p['q0']:p['q0'] + p['nq']].T
        in_maps.append({
            "possrc": possrc, "posqry": posqry, "fsrc": fsrc, "qlocf": qlocf,
            "g2": G2, "featsT": ftT, "denw": dense_w,
            "denb": dense_b.reshape(COUT, 1).astype(np.float32),
        })

    key = (NCHP, NQ)
    if key not in _COMPILED:
        _COMPILED[key] = _build_bass(NCHP, NQ)
    nc = _COMPILED[key]

    res = run_bass_kernel_spmd(nc, in_maps, core_ids=list(range(NCORES)))

    ans_conv = np.zeros((N, COUT), np.float32)
    ans_dense = np.zeros((N, COUT), np.float32)
    for c, p in enumerate(plans):
        outT = res.results[c]["outconvT"]
        for ci, (bA, bB) in enumerate(p['chunks']):
            for half, b in enumerate((bA, bB)):
                if b is None:
                    continue
                cols = ci * 16 + half * 8
                ans_conv[8 * b:8 * b + 8] = outT[:, cols:cols + 8].T
        dT = res.results[c]["outdenseT"]
        ans_dense[p['q0']:p['q0'] + p['nq']] = dT[:, 0:p['nq']].T
    return ans_conv, ans_dense



# revision 10
# speedup vs baseline: 2.2577x; 2.2577x over previous
"""Trainium2 Bass kernel for nn_ContinuousConvolutionBlock (gnn_message_passing).

Strategy (per sharding hint: partition points across 8 cores; each core owns its
queries' scatter-reduce and tap-GEMM; filter + dense weights replicated):

Host side (index plumbing / input marshalling only — zero FLOPs):
  - qry_idx is sorted; queries are grouped into 8-query blocks, blocks paired
    into 128-edge-slot "chunks" (two-pointer bin packing, ~3% padding).
  - Consecutive block ranges are assigned to the 8 cores; per-core per-slot
    payload arrays (pos[src], pos[qry], feats[src] (bf16), local query id) are
    marshalled on host and DMA'd in dense [128 x NCH x k] layout.

Device side (all FLOP-bearing compute):
  - Geometry: ball->cube volume-preserving map (DVE arithmetic + ACT
    sqrt/arctan/sign/abs) on UNSCALED relative coords (map is linear in scale,
    folded into the grid transform), then trilinear corner weights via the
    hat function w[ax] = relu(1 - |g - ax|), duplicated x2 along the tap axis
    (bf16 "dup-pair" packing) so downstream DVE ops hit the 2x packed mode.
  - Scatter-reduce as factored matmul per 128-slot chunk: with
    R[slot,(ax,c)] = w4x (x) feats  (bf16) and
    L[slot,(t,hq)]  = zy (x) Qoh    (bf16, t=(az,ay), hq = query-in-chunk),
    PE computes A^T[(ax,c),(t,hq)] = R^T @ L per chunk into PSUM.
  - PSUM->SBUF cast-copies re-arrange A into at[(ax,c), (t, chunk, hq)] bf16
    so each tap-GEMM rhs slice is fully contiguous.
  - Tap-GEMM: out^T += G_t^T @ at[:, t-slice], accumulated over 16 t in PSUM.
    G is the filter regrouped on host (pure relayout, replicated to all cores).
  - Dense branch: out_dense^T = dense_w^T @ feats^T (+bias via ACT) on PE.
  Outputs are produced transposed ([64, nq]); host transposes/reorders back.
"""
import sys
import os
sys.path.insert(0, '/opt/trn_rl_repo')
import numpy as np
import ml_dtypes

N = 30000
CIN = 32
COUT = 64
KS = 4
EXTENT = 0.08
NCORES = 8
NBLK = N // 8  # 3750 eight-query blocks

BF16 = ml_dtypes.bfloat16

_COMPILED = {}


# ----------------------------------------------------------------------------
# Host planning
# ----------------------------------------------------------------------------
def _plan(qry_idx):
    deg = np.bincount(qry_idx, minlength=N)
    bsz = deg.reshape(NBLK, 8).sum(1)
    bstart = np.concatenate([[0], np.cumsum(bsz)]).astype(np.int64)
    per = [NBLK // NCORES + (1 if c < NBLK % NCORES else 0) for c in range(NCORES)]
    b0 = np.concatenate([[0], np.cumsum(per)]).astype(np.int64)
    plans = []
    for c in range(NCORES):
        blocks = list(range(b0[c], b0[c + 1]))
        asc = sorted(blocks, key=lambda b: bsz[b])
        chunks = []
        lo, hi = 0, len(asc) - 1
        while lo <= hi:
            if lo == hi:
                chunks.append((asc[hi], None)); break
            if bsz[asc[hi]] + bsz[asc[lo]] <= 128:
                chunks.append((asc[hi], asc[lo])); hi -= 1; lo += 1
            else:
                chunks.append((asc[hi], None)); hi -= 1
        plans.append(dict(blocks=blocks, chunks=chunks, q0=int(8 * b0[c]),
                          nq=int(8 * (b0[c + 1] - b0[c]))))
    return plans, bstart, bsz


def _pack_core(plan_c, bstart, pos, feats, qry_idx, src_idx, NCHP):
    """Build per-slot payload arrays in [128, NCHP, k] layout."""
    possrc = np.zeros((128, NCHP, 4), np.float32)
    posqry = np.zeros((128, NCHP, 4), np.float32)
    fsrc = np.zeros((128, NCHP, CIN), BF16)
    qlocf = np.full((128, NCHP), -1.0, np.float32)
    for ci, (bA, bB) in enumerate(plan_c['chunks']):
        s = 0
        for half, b in enumerate((bA, bB)):
            if b is None:
                continue
            e0, e1 = int(bstart[b]), int(bstart[b + 1])
            n = e1 - e0
            sl = slice(s, s + n)
            possrc[sl, ci, 0:3] = pos[src_idx[e0:e1]]
            posqry[sl, ci, 0:3] = pos[qry_idx[e0:e1]]
            fsrc[sl, ci, :] = feats[src_idx[e0:e1]].astype(BF16)
            qlocf[sl, ci] = (qry_idx[e0:e1] - 8 * b) + 8 * half
            s += n
    return possrc, posqry, fsrc, qlocf


# ----------------------------------------------------------------------------
# Device kernel
# ----------------------------------------------------------------------------
def _build_bass(NCHP, NQ):
    import concourse.bass as bass
    import concourse.tile as tile
    from concourse import bacc, mybir
    from concourse.bass import AP

    f32 = mybir.dt.float32
    bf16 = mybir.dt.bfloat16
    i32 = mybir.dt.int32
    ALU = mybir.AluOpType
    ACT = mybir.ActivationFunctionType
    EPS = 1e-12
    F4PI = float(4.0 / np.pi)
    GSCL = float(1.5 * 2.0 / EXTENT)  # grid scale folded: g = GSCL*m + 1.5

    nc = bacc.Bacc("TRN2", target_bir_lowering=False, debug=False)

    possrc = nc.dram_tensor("possrc", (128, NCHP, 4), f32, kind="ExternalInput")
    posqry = nc.dram_tensor("posqry", (128, NCHP, 4), f32, kind="ExternalInput")
    fsrc = nc.dram_tensor("fsrc", (128, NCHP, CIN), bf16, kind="ExternalInput")
    qlocf = nc.dram_tensor("qlocf", (128, NCHP), f32, kind="ExternalInput")
    g2 = nc.dram_tensor("g2", (128, 16 * 64), bf16, kind="ExternalInput")
    featsT = nc.dram_tensor("featsT", (CIN, NQ), bf16, kind="ExternalInput")
    denw = nc.dram_tensor("denw", (CIN, COUT), bf16, kind="ExternalInput")
    denb = nc.dram_tensor("denb", (COUT, 1), f32, kind="ExternalInput")

    outconvT = nc.dram_tensor("outconvT", (COUT, NQ), f32, kind="ExternalOutput")
    outdenseT = nc.dram_tensor("outdenseT", (COUT, NQ), f32, kind="ExternalOutput")

    W = NCHP
    NGRP = NCHP // 16
    G0 = (NGRP + 1) // 2
    HALVES = [(0, G0), (G0, NGRP)]

    with tile.TileContext(nc) as tc:
        with tc.tile_pool(name="inp", bufs=1) as inp, \
             tc.tile_pool(name="geo", bufs=1) as geo, \
             tc.tile_pool(name="tmp", bufs=1) as tmp, \
             tc.tile_pool(name="lr", bufs=2) as lrp, \
             tc.tile_pool(name="at", bufs=2) as atp, \
             tc.tile_pool(name="outp", bufs=4) as outp, \
             tc.tile_pool(name="ps1", bufs=3, space="PSUM") as ps1, \
             tc.tile_pool(name="ps2", bufs=2, space="PSUM") as ps2:

            # ---------------- input DMAs ----------------
            t_ps = inp.tile([128, W, 4], f32)
            t_pq = inp.tile([128, W, 4], f32)
            t_f = inp.tile([128, W, CIN], bf16)
            t_ql = inp.tile([128, W], f32)
            t_g2 = inp.tile([128, 16 * 64], bf16)
            t_ftT = inp.tile([CIN, NQ], bf16)
            t_dw = inp.tile([CIN, COUT], bf16)
            t_db = inp.tile([COUT, 1], f32)
            nc.sync.dma_start(t_ps[:], possrc[:])
            nc.sync.dma_start(t_pq[:], posqry[:])
            nc.sync.dma_start(t_ql[:], qlocf[:])
            nc.sync.dma_start(t_ftT[:], featsT[:])
            nc.sync.dma_start(t_dw[:], denw[:])
            nc.sync.dma_start(t_db[:], denb[:])
            nc.sync.dma_start(t_f[:], fsrc[:])
            nc.sync.dma_start(t_g2[:], g2[:])

            # iota constants
            io16i = tmp.tile([128, 16], i32)
            nc.gpsimd.iota(io16i[:], pattern=[[1, 16]], base=0, channel_multiplier=0)
            io16 = geo.tile([128, 16], f32)
            nc.vector.tensor_copy(io16[:], io16i[:])
            # c4m = [0,0,1,1,2,2,3,3] - 1.5  (dup-pair tap offsets)
            c4di = tmp.tile([128, 8], i32)
            nc.gpsimd.iota(c4di[:], pattern=[[1, 4], [0, 2]], base=0,
                           channel_multiplier=0)
            c4m = geo.tile([128, 8], f32)
            nc.vector.tensor_copy(c4m[:], c4di[:])
            nc.vector.tensor_scalar(c4m[:], c4m[:], 1.5, None, op0=ALU.subtract)

            # ---------------- dense branch (bf16 matmul, runs first) --------
            NSEG = (NQ + 511) // 512
            for s in range(NSEG):
                j0 = s * 512
                j1 = min(NQ, j0 + 512)
                pd = ps2.tile([COUT, 512], f32, space="PSUM", tag="po")
                nc.tensor.matmul(
                    out=pd[:, 0:j1 - j0],
                    lhsT=t_dw[:],
                    rhs=t_ftT[:, j0:j1],
                    start=True, stop=True)
                odt = outp.tile([COUT, 512], f32, tag="odst")
                db = t_db[:, 0:1]
                nc.scalar.activation(odt[:, 0:j1 - j0], pd[:, 0:j1 - j0],
                                     ACT.Identity, bias=db, scale=1.0)
                nc.sync.dma_start(outdenseT[:, j0:j1], odt[:, 0:j1 - j0])

            # ---------------- per-half geometry + pipeline ----------------
            # temp tile machinery (tags reused across halves)
            _tn = [0]
            _free_tags = []
            _tag_of = {}
            _seq = [0]

            def T(shape, dt_=f32):
                key = tuple(shape) + (dt_,)
                pool_tags = _free_tags
                for i, (tg, k) in enumerate(pool_tags):
                    if k == key:
                        pool_tags.pop(i)
                        break
                else:
                    _tn[0] += 1
                    tg = f"t{_tn[0]}"
                _seq[0] += 1
                t = tmp.tile(list(shape), dt_, name=f"{tg}_u{_seq[0]}", tag=tg)
                _tag_of[id(t)] = (tg, key)
                return t

            def F(*ts):
                for t in ts:
                    _free_tags.append(_tag_of.pop(id(t)))

            TT = nc.vector.tensor_tensor
            TS = nc.vector.tensor_scalar
            STT = nc.vector.scalar_tensor_tensor

            # delayed tap-GEMM state for PE software pipelining
            pend = []  # list of (at_tile, global_group_idx)

            def flush_tap():
                if not pend:
                    return
                at_t, gg = pend.pop(0)
                po = ps2.tile([COUT, 512], f32, space="PSUM", tag="po")
                for t in range(16):
                    nc.tensor.matmul(
                        out=po[:, 0:256],
                        lhsT=t_g2[:, t * 64:(t + 1) * 64],
                        rhs=at_t[:, t * 256:(t + 1) * 256],
                        start=(t == 0), stop=(t == 15))
                ost = outp.tile([COUT, 256], f32, tag="ocst")
                nc.scalar.copy(ost[:], po[:, 0:256])
                nc.sync.dma_start(outconvT[:, gg * 256:(gg + 1) * 256], ost[:])

            for (g_lo, g_hi) in HALVES:
                c0 = g_lo * 16          # first chunk of this half
                Wh = (g_hi - g_lo) * 16  # chunks in this half

                # ---------------- geometry on [128, Wh] ----------------
                # unscaled relative coords r = ps - pq (scale folded into GSCL)
                rs = T((128, Wh, 3))
                TT(out=rs[:], in0=t_ps[:, c0:c0 + Wh, 0:3],
                   in1=t_pq[:, c0:c0 + Wh, 0:3], op=ALU.subtract)
                x, y, z = rs[:, :, 0], rs[:, :, 1], rs[:, :, 2]

                sq3 = T((128, Wh, 3))
                TT(out=sq3[:], in0=rs[:], in1=rs[:], op=ALU.mult)
                x2, y2, z2 = sq3[:, :, 0], sq3[:, :, 1], sq3[:, :, 2]
                xy2 = T((128, Wh))
                TT(out=xy2[:], in0=x2, in1=y2, op=ALU.add)

                norm = T((128, Wh))
                sq = T((128, Wh))
                TT(out=sq[:], in0=xy2[:], in1=z2, op=ALU.add)
                nc.scalar.activation(norm[:], sq[:], ACT.Sqrt)
                F(sq)
                nxy = T((128, Wh))
                nc.scalar.activation(nxy[:], xy2[:], ACT.Sqrt)

                # pole = 1.25*z2 > xy2
                pole = T((128, Wh))
                STT(out=pole[:], in0=z2, scalar=1.25, in1=xy2[:],
                    op0=ALU.mult, op1=ALU.is_gt)
                F(sq3, xy2)

                # s1 = sqrt(3*norm / (norm + |z| + eps))
                azn = T((128, Wh))
                nc.scalar.activation(azn[:], z, ACT.Abs)
                den1 = T((128, Wh))
                STT(out=den1[:], in0=azn[:], scalar=EPS, in1=norm[:],
                    op0=ALU.add, op1=ALU.add)
                rd1 = T((128, Wh))
                nc.vector.reciprocal_approx_fast(rd1[:], den1[:])
                t1s = T((128, Wh))
                STT(out=t1s[:], in0=norm[:], scalar=3.0, in1=rd1[:],
                    op0=ALU.mult, op1=ALU.mult)
                s1 = T((128, Wh))
                nc.scalar.activation(s1[:], t1s[:], ACT.Sqrt)
                F(azn, den1, rd1, t1s)

                # s2 = norm / (nxy + eps)
                den2 = T((128, Wh))
                TS(den2[:], nxy[:], EPS, None, op0=ALU.add)
                rd2 = T((128, Wh))
                nc.vector.reciprocal_approx_fast(rd2[:], den2[:])
                s2 = T((128, Wh))
                TT(out=s2[:], in0=norm[:], in1=rd2[:], op=ALU.mult)
                F(nxy, den2, rd2)

                # wq = s2 + pole*(s1-s2)
                d12 = T((128, Wh))
                TT(out=d12[:], in0=s1[:], in1=s2[:], op=ALU.subtract)
                pw = T((128, Wh))
                TT(out=pw[:], in0=pole[:], in1=d12[:], op=ALU.mult)
                wq = T((128, Wh))
                TT(out=wq[:], in0=s2[:], in1=pw[:], op=ALU.add)
                F(s1, s2, d12, pw)

                xc = T((128, Wh))
                TT(out=xc[:], in0=x, in1=wq[:], op=ALU.mult)
                yc = T((128, Wh))
                TT(out=yc[:], in0=y, in1=wq[:], op=ALU.mult)
                F(wq)

                # zc = 1.5z + pole*(sign(z)*norm - 1.5z)
                sgz = T((128, Wh))
                nc.scalar.activation(sgz[:], z, ACT.Sign)
                zcp = T((128, Wh))
                TT(out=zcp[:], in0=sgz[:], in1=norm[:], op=ALU.mult)
                zce = T((128, Wh))
                TS(zce[:], z, 1.5, None, op0=ALU.mult)
                dz = T((128, Wh))
                TT(out=dz[:], in0=zcp[:], in1=zce[:], op=ALU.subtract)
                pz = T((128, Wh))
                TT(out=pz[:], in0=pole[:], in1=dz[:], op=ALU.mult)
                zc = T((128, Wh))
                TT(out=zc[:], in0=zce[:], in1=pz[:], op=ALU.add)
                F(sgz, zcp, zce, dz, pz, pole, norm, rs)

                # cylinder -> cube
                xc2 = T((128, Wh))
                TT(out=xc2[:], in0=xc[:], in1=xc[:], op=ALU.mult)
                yc2 = T((128, Wh))
                TT(out=yc2[:], in0=yc[:], in1=yc[:], op=ALU.mult)
                sqxy = T((128, Wh))
                TT(out=sqxy[:], in0=xc2[:], in1=yc2[:], op=ALU.add)
                nrm = T((128, Wh))
                nc.scalar.activation(nrm[:], sqxy[:], ACT.Sqrt)
                F(xc2, yc2, sqxy)

                axc = T((128, Wh))
                nc.scalar.activation(axc[:], xc[:], ACT.Abs)
                ayc = T((128, Wh))
                nc.scalar.activation(ayc[:], yc[:], ACT.Abs)
                abr = T((128, Wh))
                TT(out=abr[:], in0=ayc[:], in1=axc[:], op=ALU.is_le)

                mx = T((128, Wh))
                TS(mx[:], axc[:], EPS, None, op0=ALU.is_lt)
                sfx = T((128, Wh))
                TT(out=sfx[:], in0=xc[:], in1=mx[:], op=ALU.add)
                my = T((128, Wh))
                TS(my[:], ayc[:], EPS, None, op0=ALU.is_lt)
                sfy = T((128, Wh))
                TT(out=sfy[:], in0=yc[:], in1=my[:], op=ALU.add)
                F(axc, ayc, mx, my)

                rsx = T((128, Wh))
                nc.vector.reciprocal_approx_fast(rsx[:], sfx[:])
                rsy = T((128, Wh))
                nc.vector.reciprocal_approx_fast(rsy[:], sfy[:])
                ratx = T((128, Wh))
                TT(out=ratx[:], in0=xc[:], in1=rsy[:], op=ALU.mult)
                raty = T((128, Wh))
                TT(out=raty[:], in0=yc[:], in1=rsx[:], op=ALU.mult)
                at1 = T((128, Wh))
                nc.scalar.activation(at1[:], ratx[:], ACT.Arctan)
                at2 = T((128, Wh))
                nc.scalar.activation(at2[:], raty[:], ACT.Arctan)
                F(sfx, sfy, rsx, rsy, ratx, raty)

                sgx = T((128, Wh))
                nc.scalar.activation(sgx[:], xc[:], ACT.Sign)
                sgy = T((128, Wh))
                nc.scalar.activation(sgy[:], yc[:], ACT.Sign)
                tmpa = T((128, Wh))
                TT(out=tmpa[:], in0=sgx[:], in1=nrm[:], op=ALU.mult)
                tmpb = T((128, Wh))
                TT(out=tmpb[:], in0=sgy[:], in1=nrm[:], op=ALU.mult)
                F(sgx, sgy, nrm, xc, yc)

                # xo = xoe + abr*(tmpa - xoe), xoe = tmpb*F4PI*at1
                xoe = T((128, Wh))
                STT(out=xoe[:], in0=at1[:], scalar=F4PI, in1=tmpb[:],
                    op0=ALU.mult, op1=ALU.mult)
                dxo = T((128, Wh))
                TT(out=dxo[:], in0=tmpa[:], in1=xoe[:], op=ALU.subtract)
                adx = T((128, Wh))
                TT(out=adx[:], in0=abr[:], in1=dxo[:], op=ALU.mult)
                xo = T((128, Wh))
                TT(out=xo[:], in0=xoe[:], in1=adx[:], op=ALU.add)
                # yo = tmpb + abr*(yoe - tmpb), yoe = tmpa*F4PI*at2
                yoe = T((128, Wh))
                STT(out=yoe[:], in0=at2[:], scalar=F4PI, in1=tmpa[:],
                    op0=ALU.mult, op1=ALU.mult)
                dyo = T((128, Wh))
                TT(out=dyo[:], in0=yoe[:], in1=tmpb[:], op=ALU.subtract)
                ady = T((128, Wh))
                TT(out=ady[:], in0=abr[:], in1=dyo[:], op=ALU.mult)
                yo = T((128, Wh))
                TT(out=yo[:], in0=tmpb[:], in1=ady[:], op=ALU.add)
                F(at1, at2, xoe, dxo, adx, yoe, dyo, ady, tmpa, tmpb, abr)

                # ---------------- hat weights, dup-pair packed bf16 --------
                # w4m2[slot, 2*ax+r] = relu(1 - |GSCL*m + 1.5 - ax|)
                def hat_w4(m_ap, w4_t, deng):
                    d = T((128, Wh, 8))
                    deng(out=d[:],
                         in0=AP(m_ap.tensor, m_ap.offset,
                                [m_ap.ap[0], [1, Wh], [0, 8]]),
                         scalar=GSCL,
                         in1=AP(c4m.tensor, c4m[:].offset,
                                [c4m[:].ap[0], [0, Wh], [1, 8]]),
                         op0=ALU.mult, op1=ALU.subtract)
                    a = T((128, Wh, 8))
                    nc.scalar.activation(a[:], d[:], ACT.Abs)
                    nc.scalar.activation(w4_t[:], a[:], ACT.Relu,
                                         bias=1.0, scale=-1.0)
                    F(d, a)

                w4x2 = geo.tile([128, Wh, 8], bf16, tag=f"w4x2_{g_lo}")
                w4y2 = T((128, Wh, 8), bf16)
                w4z2 = T((128, Wh, 8), bf16)
                hat_w4(xo[:], w4x2, STT)
                hat_w4(yo[:], w4y2, STT)
                hat_w4(zc[:], w4z2, STT)
                F(xo, yo, zc)

                # zy2[slot, az*8 + ay*2 + r] = w4z[az]*w4y[ay]  (bf16 2x)
                zy2 = geo.tile([128, Wh, 32], bf16, tag=f"zy2_{g_lo}")
                for az in range(4):
                    zslc = w4z2[:, :, 2 * az:2 * az + 2]
                    TT(out=AP(zy2.tensor, zy2[:].offset + az * 8,
                              [zy2[:].ap[0], [32, Wh], [1, 8]]),
                       in0=AP(w4z2.tensor, zslc.offset,
                              [zslc.ap[0], [8, Wh], [0, 4], [1, 2]]),
                       in1=AP(w4y2.tensor, w4y2[:].offset,
                              [w4y2[:].ap[0], [8, Wh], [1, 8]]),
                       op=ALU.mult)
                F(w4y2, w4z2)

                # qoh[slot, hq] = (qloc == hq)  (bf16 out)
                qoh = geo.tile([128, Wh, 16], bf16, tag=f"qoh_{g_lo}")
                qslc = t_ql[:, c0:c0 + Wh]
                TT(
                    out=qoh[:],
                    in0=AP(t_ql.tensor, qslc.offset,
                           [qslc.ap[0], [1, Wh], [0, 16]]),
                    in1=AP(io16.tensor, io16[:].offset,
                           [io16[:].ap[0], [0, Wh], [1, 16]]),
                    op=ALU.is_equal)

                # ---------------- per-group builds + matmuls ----------------
                for g in range(g_lo, g_hi):
                    gl = g - g_lo
                    # L[slot, ci*256 + t*16 + hq] = zy2 * qoh  (bf16 2x)
                    L = lrp.tile([128, 4096], bf16, tag="L")
                    TT(out=AP(L.tensor, L[:].offset,
                              [L[:].ap[0], [16, 256], [1, 16]]),
                       in0=AP(zy2.tensor, zy2[:].offset + gl * 16 * 32,
                              [zy2[:].ap[0], [2, 256], [0, 8], [1, 2]]),
                       in1=AP(qoh.tensor, qoh[:].offset + gl * 16 * 16,
                              [qoh[:].ap[0], [16, 16], [0, 16], [1, 16]]),
                       op=ALU.mult)
                    # R[slot, ci*128 + ax*32 + c] = w4x2 * feats  (bf16 2x)
                    R = lrp.tile([128, 2048], bf16, tag="R")
                    TT(out=AP(R.tensor, R[:].offset,
                              [R[:].ap[0], [32, 64], [1, 32]]),
                       in0=AP(w4x2.tensor, w4x2[:].offset + gl * 16 * 8,
                              [w4x2[:].ap[0], [2, 64], [0, 16], [1, 2]]),
                       in1=AP(t_f.tensor, t_f[:].offset + g * 16 * CIN,
                              [t_f[:].ap[0], [32, 16], [0, 4], [1, 32]]),
                       op=ALU.mult)

                    # stage-1: 4 quads x 4 matmuls into [128,1024] PSUM
                    at_t = atp.tile([128, 4096], bf16, tag="at")
                    for q in range(4):
                        ps_t = ps1.tile([128, 1024], f32, space="PSUM", tag="s1")
                        for k in range(4):
                            ci = q * 4 + k
                            nc.tensor.matmul(
                                out=ps_t[:, k * 256:(k + 1) * 256],
                                lhsT=R[:, ci * 128:(ci + 1) * 128],
                                rhs=L[:, ci * 256:(ci + 1) * 256],
                                start=True, stop=True)
                        # copy quad into at[(t, cig, hq)] (cast to bf16)
                        dst = AP(at_t.tensor, at_t[:].offset + q * 4 * 16,
                                 [at_t[:].ap[0], [16, 4], [256, 16], [1, 16]])
                        src = AP(ps_t.tensor, ps_t[:].offset,
                                 [ps_t[:].ap[0], [256, 4], [16, 16], [1, 16]])
                        if q == 1:
                            nc.vector.tensor_copy(dst, src)
                        else:
                            nc.scalar.copy(dst, src)
                    pend.append((at_t, g))
                    if len(pend) > 1:
                        flush_tap()
            while pend:
                flush_tap()

    nc.compile()
    return nc


# ----------------------------------------------------------------------------
# Host-side input prep (shared by kernel() and test.py's profile path)
# ----------------------------------------------------------------------------
def _prepare(feats, pos, filt, dense_w, dense_b, src_idx, qry_idx):
    feats = np.ascontiguousarray(np.asarray(feats, np.float32))
    pos = np.ascontiguousarray(np.asarray(pos, np.float32))
    filt = np.asarray(filt, np.float32)
    dense_w = np.asarray(dense_w, np.float32)
    dense_b = np.asarray(dense_b, np.float32)
    src_idx = np.asarray(src_idx).astype(np.int64)
    qry_idx = np.asarray(qry_idx).astype(np.int64)

    plans, bstart, bsz = _plan(qry_idx)
    NCH = max(len(p['chunks']) for p in plans)
    NCHP = ((NCH + 15) // 16) * 16
    NQ = NCHP * 16

    # filter regroup: G2[ax*32+c, t*64+o] = filt[az, ay, ax, c, o], t = az*4+ay
    G2 = np.zeros((128, 16 * 64), np.float32)
    for az in range(4):
        for ay in range(4):
            t = az * 4 + ay
            for ax in range(4):
                G2[ax * 32:(ax + 1) * 32, t * 64:(t + 1) * 64] = filt[az, ay, ax]
    G2 = G2.astype(BF16)

    in_maps = []
    for c, p in enumerate(plans):
        possrc, posqry, fsrc, qlocf = _pack_core(p, bstart, pos, feats,
                                                 qry_idx, src_idx, NCHP)
        ftT = np.zeros((CIN, NQ), BF16)
        ftT[:, 0:p['nq']] = feats[p['q0']:p['q0'] + p['nq']].T.astype(BF16)
        in_maps.append({
            "possrc": possrc, "posqry": posqry, "fsrc": fsrc, "qlocf": qlocf,
            "g2": G2, "featsT": ftT, "denw": dense_w.astype(BF16),
            "denb": dense_b.reshape(COUT, 1).astype(np.float32),
        })
    return plans, in_maps, NCHP, NQ


# ----------------------------------------------------------------------------
# Entry point
# ----------------------------------------------------------------------------
def kernel(feats, pos, filt, dense_w, dense_b, src_idx, qry_idx):
    from concourse.bass_utils import run_bass_kernel_spmd

    plans, in_maps, NCHP, NQ = _prepare(feats, pos, filt, dense_w, dense_b,
                                        src_idx, qry_idx)

    key = (NCHP, NQ)
    if key not in _COMPILED:
        _COMPILED[key] = _build_bass(NCHP, NQ)
    nc = _COMPILED[key]

    res = run_bass_kernel_spmd(nc, in_maps, core_ids=list(range(NCORES)))

    ans_conv = np.zeros((N, COUT), np.float32)
    ans_dense = np.zeros((N, COUT), np.float32)
    for c, p in enumerate(plans):
        outT = res.results[c]["outconvT"]
        for ci, (bA, bB) in enumerate(p['chunks']):
            for half, b in enumerate((bA, bB)):
                if b is None:
                    continue
                cols = ci * 16 + half * 8
                ans_conv[8 * b:8 * b + 8] = outT[:, cols:cols + 8].T
        dT = res.results[c]["outdenseT"]
        ans_dense[p['q0']:p['q0'] + p['nq']] = dT[:, 0:p['nq']].T
    return ans_conv, ans_dense
